# revision 49
# baseline (speedup 1.0000x reference)
"""BoundaryLoss TRN2 kernel — 8-core SPMD Bass/Tile implementation (v3).

Data-parallel over batch B=8, one batch per NeuronCore.

v3 changes over v2 (trace-driven):
  * All bulk DMAs (emb chunks, memory banks) issue immediately at program
    start, spread across tensor/gpsimd/scalar engine queues so the sync
    queue only carries the latency-critical counts path (preds halves,
    fsss, consts, pid).
  * GpSimd no longer does the per-chunk mw multiplies (which delayed its
    collective servicing); it only does half of the channel reduce_max
    early, then sits at the two collective_compute instructions.
  * preds arrives in two halves; vector and gpsimd each reduce_max one
    half concurrently -> counts AllGather triggers ~6us earlier.
  * Norm phase: squares run bf16 tensor_tensor on vector (2x DVE mode)
    for most chunks / scalar activation for the rest; the per-tile
    sum-of-squares reduce outputs bf16 (enables 2x DVE mode).  A dummy
    Sqrt at t=0 preloads the activation table so no mid-stream reload.
  * Masks live in one [128, 3*128] tile -> one row-sum reduce.
  * Post-AG path: offsets+totals in one matmul, banks folded into the
    per-core AG2 payload (pid==0 mask), AG2 carries [128,2] only.

Self-contained: hardcodes all shapes for the nn_BoundaryLoss problem
(B=8, C=21, H=W=512, D=128, h=w=128, MEM=1000).
"""

import ml_dtypes
import numpy as np

import concourse.bass as bass
import concourse.bacc as bacc
import concourse.mybir as mybir
import concourse.tile as tile
from concourse.bass_utils import run_bass_kernel_spmd
from concourse.tile_rust import add_dep_helper

F32 = mybir.dt.float32
BF16 = mybir.dt.bfloat16
I32 = mybir.dt.int32
U32 = mybir.dt.uint32
Alu = mybir.AluOpType
AX = mybir.AxisListType
ACT = mybir.ActivationFunctionType

N_CORES = 8
CORE_IDS = list(range(N_CORES))

# The tile scheduling simulator is single-core: it never sees the remote
# semaphore increments produced by peer cores' remote DMAs, so a wait on
# them would be flagged as a deadlock.  Pre-seed those semaphores in the
# simulator only — the emitted hardware program is unchanged.
_P2P_PRESEED = []  # list of (SemaphoreHandle, count)


def _patch_coresim():
    import concourse.tile as _tile
    from concourse.bass import create_sync_update as _csu
    if getattr(_tile, "_p2p_patched", False):
        return
    _orig = _tile.CoreSim

    class _P2PCoreSim(_orig):
        def simulate(self, *a, **k):
            for sem, cnt in _P2P_PRESEED:
                try:
                    self.update_semaphore(
                        _csu(sem, cnt, None, True), instruction=None)
                except Exception:
                    pass
            return super().simulate(*a, **k)

    _tile.CoreSim = _P2PCoreSim
    _tile._p2p_patched = True
P = 128          # partitions / feature spatial dim
NCH = 21         # pred classes
D = 128          # embedding dim
MEM = 1000
K_ANC, K_POS, K_NEG = MEM // 10, MEM // 3, MEM // 3
MARGIN = 0.2
NCHUNK = 8       # embedding chunks per core (16 tiles of 128 positions each)
TPC = 16         # tiles per chunk


def _build():
    nc = bacc.Bacc("TRN2", target_bir_lowering=False, debug=False,
                   num_devices=N_CORES)

    preds_in = nc.declare_dram_parameter("p_preds", [P, P * NCH], BF16, isOutput=False)
    fsss_in = nc.declare_dram_parameter("p_fsss", [P, P], BF16, isOutput=False)
    emb_in = nc.declare_dram_parameter("p_emb", [NCHUNK, P, TPC * D], BF16, isOutput=False)
    pmem_in = nc.declare_dram_parameter("p_pmem", [P, 8 * D], BF16, isOutput=False)
    nmem_in = nc.declare_dram_parameter("p_nmem", [P, 8 * D], BF16, isOutput=False)
    cst_in = nc.declare_dram_parameter("c_all", [P, 448], F32, isOutput=False)

    out_t = nc.declare_dram_parameter("out", [P, 3], F32, isOutput=True)

    with tile.TileContext(nc, num_cores=N_CORES) as tc:
        with (
            tc.tile_pool(name="sb", bufs=1) as sb,
            tc.tile_pool(name="ps", bufs=1, space="PSUM") as ps,
            tc.tile_pool(name="pst", bufs=2, space="PSUM") as pst,
            tc.tile_pool(name="dr", bufs=1, space="DRAM") as dr,
        ):
            # ============== input DMAs ==============
            # sync queue: counts-critical path only
            cst = sb.tile([P, 448], F32)
            nc.sync.dma_start(out=cst[:], in_=cst_in[:])
            preds_sb = sb.tile([P, P * NCH], BF16)
            HW = 64 * NCH
            nc.sync.dma_start(out=preds_sb[:, 0:HW], in_=preds_in[:, 0:HW])
            p1_dma = nc.sync.dma_start(out=preds_sb[:, HW:2 * HW],
                                       in_=preds_in[:, HW:2 * HW])
            fsss_sb = sb.tile([P, P], BF16)
            nc.sync.dma_start(out=fsss_sb[:], in_=fsss_in[:])
            pid_sb = sb.tile([1, 1], U32)
            nc.sync.dma_start(out=pid_sb[:], in_=nc.partition_id_tensor[0:1, 0:1])

            # bulk DMA tiles; the dma_starts are issued after the counts
            # DMA so the tiny AllGather input never queues behind megabytes
            # of embedding descriptors (norm still finishes well inside the
            # collective's bootstrap window).
            emb_sb = [sb.tile([P, TPC * D], BF16, name=f"ech{c}")
                      for c in range(NCHUNK)]
            pmem = sb.tile([P, 8 * D], BF16)
            nmem = sb.tile([P, 8 * D], BF16)

            ident = cst[:, 0:128]
            lincl = cst[:, 128:256]
            iotam = cst[:, 256:264]
            onescol = cst[:, 264:265]
            ones8 = cst[0:8, 264:265]
            onesrow = cst[0:1, 265:393]
            ones8r = cst[0:1, 265:273]
            iota8 = cst[0:8, 393:394]
            k3row = cst[0:1, 394:397]
            margin0 = cst[0:1, 408:409]
            x24 = cst[0:8, 416:440]          # x24[s, i*3+x] = s ^ i

            # scalar: preload activation table (Sqrt) while DMAs stream
            dummy = sb.tile([1, 1], F32)
            nc.scalar.activation(out=dummy[:], in_=cst[0:1, 264:265], func=ACT.Sqrt)
            identb = sb.tile([P, P], BF16)
            nc.scalar.copy(identb[:], ident)
            linclb = sb.tile([P, P], BF16)   # inclusive upper-tri [c<=p]
            nc.scalar.copy(linclb[:], lincl)
            onescolb = sb.tile([P, 1], BF16)
            nc.scalar.copy(onescolb[:], onescol)

            # ============== mask phase ==============
            preds_re = preds_sb[:].rearrange("p (w c) -> p w c", c=NCH)
            mx = sb.tile([P, P], BF16)
            nc.vector.reduce_max(out=mx[:, 0:64], in_=preds_re[:, 0:64, 1:NCH],
                                 axis=AX.X)
            nc.vector.reduce_max(out=mx[:, 64:128], in_=preds_re[:, 64:128, 1:NCH],
                                 axis=AX.X)
            pm = sb.tile([P, P], BF16)  # pred_mask
            nc.vector.tensor_tensor(out=pm[:], in0=mx[:], in1=preds_re[:, :, 0],
                                    op=Alu.is_gt)
            e0 = sb.tile([P, P], BF16)
            nc.vector.tensor_scalar(out=e0[:], in0=fsss_sb[:], scalar1=0.0,
                                    scalar2=None, op0=Alu.is_equal)
            e255 = sb.tile([P, P], BF16)
            nc.vector.tensor_scalar(out=e255[:], in0=fsss_sb[:], scalar1=255.0,
                                    scalar2=None, op0=Alu.is_equal)
            s01 = sb.tile([P, P], BF16)
            nc.vector.tensor_tensor(out=s01[:], in0=e0[:], in1=e255[:], op=Alu.add)
            fm = sb.tile([P, P], BF16)  # fsss_mask
            nc.vector.tensor_scalar(out=fm[:], in0=s01[:], scalar1=-1.0,
                                    scalar2=1.0, op0=Alu.mult, op1=Alu.add)
            m3 = sb.tile([P, 3 * P], BF16)  # anchor | positive | negative
            am, pom, nm = m3[:, 0:P], m3[:, P:2 * P], m3[:, 2 * P:3 * P]
            nc.vector.tensor_tensor(out=am, in0=pm[:], in1=fm[:], op=Alu.mult)
            nc.vector.tensor_tensor(out=pom, in0=fm[:], in1=am, op=Alu.subtract)
            nc.vector.tensor_tensor(out=nm, in0=pm[:], in1=e0[:], op=Alu.mult)

            # row sums -> totals -> counts AllGather input, ASAP
            rs = sb.tile([P, 3], F32)
            rs_i = nc.vector.reduce_sum(
                out=rs[:], in_=m3[:].rearrange("p (x w) -> p x w", x=3), axis=AX.X)
            tot_ps = pst.tile([1, 3], F32, tag="smallps")
            nc.tensor.matmul(out=tot_ps[:], lhsT=onescol[:], rhs=rs[:],
                             start=True, stop=True)
            tot_sb = sb.tile([1, 4], F32)
            nc.vector.tensor_copy(tot_sb[:, 0:3], tot_ps[:])
            ag_in_d = dr.tile([1, 3], F32)
            ag_out_d = dr.tile([8, 3], F32, addr_space="Shared")
            agin_dma = nc.sync.dma_start(out=ag_in_d[:], in_=tot_sb[:, 0:3])
            nc.gpsimd.collective_compute(
                "AllGather", Alu.bypass, replica_groups=[CORE_IDS],
                ins=[ag_in_d[:]], outs=[ag_out_d[:]])
            for c in range(3):
                d = nc.sync.dma_start(out=emb_sb[c][:], in_=emb_in[c])
                add_dep_helper(d.ins, agin_dma.ins, reason="counts DMA first")
            for c in range(3, 8):
                d = nc.scalar.dma_start(out=emb_sb[c][:], in_=emb_in[c])
                add_dep_helper(d.ins, agin_dma.ins, reason="counts DMA first")
            nc.sync.dma_start(out=pmem[:], in_=pmem_in[:])
            nc.sync.dma_start(out=nmem[:], in_=nmem_in[:])
            cnts8 = sb.tile([8, 3], F32)
            nc.sync.dma_start(out=cnts8[:], in_=ag_out_d[:])

            # pid helpers: lower-tri column, [128,1] pid broadcast, pid==0
            pid_f = sb.tile([1, 1], F32)
            nc.vector.tensor_copy(pid_f[:], pid_sb[:])
            pidb_ps = pst.tile([P, 1], F32, tag="smallps")
            nc.tensor.matmul(out=pidb_ps[:], lhsT=onesrow[:], rhs=pid_f[:],
                             start=True, stop=True)
            pid0 = sb.tile([P, 1], F32)
            nc.vector.tensor_scalar(out=pid0[:], in0=pidb_ps[:], scalar1=1.0,
                                    scalar2=None, op0=Alu.is_lt)
            ltm = sb.tile([8, 2], F32)
            nc.vector.tensor_tensor(out=ltm[:, 0:1], in0=iota8[:],
                                    in1=pidb_ps[0:8, :], op=Alu.is_lt)
            nc.vector.tensor_copy(ltm[:, 1:2], ones8)

            # transposes: masks to [w, x*128+h] layout (pre-AG, PE)
            mT = sb.tile([P, 3 * P], BF16)
            for x in range(3):
                mt_ps = pst.tile([P, P], BF16, name="mt_ps", tag="smallps")
                nc.tensor.transpose(out=mt_ps[:], in_=m3[:, x * P:(x + 1) * P],
                                    identity=identb[:])
                nc.scalar.copy(mT[:, x * P:(x + 1) * P], mt_ps[:])

            # cumT[w, x*128+h] = excl[x,h] + sum_{w'<=w} mT[w', x*128+h]
            # built entirely on PE: column-prefix matmul + broadcast add.
            rsT_ps = pst.tile([1, 3 * P], F32, tag="rsT")
            nc.tensor.matmul(out=rsT_ps[:], lhsT=onescolb[:], rhs=mT[:],
                             start=True, stop=True)
            rsT_sb = sb.tile([1, 3 * P], F32)
            nc.vector.tensor_copy(rsT_sb[:], rsT_ps[:])
            excl_row = sb.tile([1, 3 * P], F32)
            nc.vector.memset(excl_row[:], 0.0)
            for x in range(3):
                nc.vector.tensor_tensor_scan(
                    out=excl_row[0:1, x * P + 1:(x + 1) * P],
                    data0=rsT_sb[0:1, x * P:x * P + P - 1],
                    data1=rsT_sb[0:1, x * P:x * P + P - 1],
                    initial=0.0, op0=Alu.add, op1=Alu.bypass)
            cumT_ps = ps.tile([P, 3 * P], F32)
            nc.tensor.matmul(out=cumT_ps[:], lhsT=linclb[:], rhs=mT[:],
                             start=True, stop=False)
            nc.tensor.matmul(out=cumT_ps[:], lhsT=onesrow[:], rhs=excl_row[:],
                             start=False, stop=True)

            # ============== embedding normalisation stream ==============
            # inv[w, h] = 1/sqrt(sum_d emb[w, h, d]^2)
            inv_all = sb.tile([P, P], F32)
            ss = sb.tile([P, P], BF16)       # per-tile sum of squares
            nrm = sb.tile([P, P], BF16)      # sqrt of ss
            sq = [sb.tile([P, TPC * D], BF16, name=f"sq{i}") for i in range(3)]
            SC_SQ = (2, 5, 7)                # chunks squared on scalar engine
            for c in range(NCHUNK):
                ech = emb_sb[c][:]
                sqc = sq[c % 3]
                if c in SC_SQ:
                    nc.scalar.square(sqc[:], ech)
                else:
                    sq_i = nc.vector.tensor_tensor(out=sqc[:], in0=ech, in1=ech,
                                                   op=Alu.mult)
                    if c == 0:
                        # counts path keeps vector priority over norm work
                        add_dep_helper(sq_i.ins, rs_i.ins, reason="counts first")
                ssc = ss[:, c * TPC:(c + 1) * TPC]
                with nc.allow_low_precision("sumsq rounded to bf16 (tol 2e-2)"):
                    nc.vector.reduce_sum(
                        out=ssc, in_=sqc[:].rearrange("p (t d) -> p t d", t=TPC),
                        axis=AX.X)
                if c == 3:
                    nc.scalar.activation(out=nrm[:, 0:64], in_=ss[:, 0:64],
                                         func=ACT.Sqrt)
                    nc.vector.reciprocal(inv_all[:, 0:64], nrm[:, 0:64])
            nc.scalar.activation(out=nrm[:, 64:128], in_=ss[:, 64:128],
                                 func=ACT.Sqrt)
            nc.vector.reciprocal(inv_all[:, 64:128], nrm[:, 64:128])

            # mw = mT * inv (pre-AG, vector; x-major layout like mT)
            mw = sb.tile([P, 3 * P], BF16)
            mw_last = None
            for x in range(3):
                mw_last = nc.vector.tensor_tensor(
                    out=mw[:, x * P:(x + 1) * P],
                    in0=mT[:, x * P:(x + 1) * P],
                    in1=inv_all[:], op=Alu.mult)

            # ============== post-AG: offsets, cutoffs ==============
            offs_ps = pst.tile([1, 3], F32, tag="smallps")
            nc.tensor.matmul(out=offs_ps[:], lhsT=ltm[:, 0:1], rhs=cnts8[:],
                             start=True, stop=True)
            gtot_ps = pst.tile([1, 3], F32, tag="smallps")
            nc.tensor.matmul(out=gtot_ps[:], lhsT=ltm[:, 1:2], rhs=cnts8[:],
                             start=True, stop=True)
            cn_in = sb.tile([1, 6], F32)
            cn_sub = nc.vector.tensor_tensor(out=cn_in[:, 0:3], in0=k3row[:],
                                             in1=offs_ps[:], op=Alu.subtract)
            # keep pre-AG work (recip/mw) ahead of the AG-blocked ops in the
            # vector queue — without this the scheduler parks the queue at
            # cn_sub and mw slips onto the post-AG critical path.
            add_dep_helper(cn_sub.ins, mw_last.ins, reason="mw pre-AG")
            nc.vector.tensor_tensor(out=cn_in[:, 3:6], in0=gtot_ps[:],
                                    in1=k3row[:], op=Alu.min)
            n3 = cn_in[:, 3:6]
            cn_ps = pst.tile([P, 6], F32, tag="smallps")
            nc.tensor.matmul(out=cn_ps[:], lhsT=onesrow[:], rhs=cn_in[:],
                             start=True, stop=True)
            cnb = sb.tile([P, 6], F32)
            nc.vector.tensor_copy(cnb[:], cn_ps[:])
            c3b, n3b = cnb[:, 0:3], cnb[:, 3:6]

            # selection weights (x-major like mT/cumT): wv = (cumT<=c3)*mw
            wv = sb.tile([P, 3 * P], BF16)
            for x in range(3):
                nc.vector.scalar_tensor_tensor(
                    out=wv[:, x * P:(x + 1) * P],
                    in0=cumT_ps[:, x * P:(x + 1) * P],
                    scalar=c3b[:, x:x + 1], in1=mw[:, x * P:(x + 1) * P],
                    op0=Alu.is_le, op1=Alu.mult)
            wv_h = wv[:].rearrange("p (x h) -> p h x", x=3)

            # ============== S3 stream: emb stationary, weights moving ==========
            s3_ps = ps.tile([P, 3], F32)
            for c in range(NCHUNK):
                ech = emb_sb[c][:]
                for tt in range(TPC):
                    t = c * TPC + tt
                    nc.tensor.matmul(
                        out=s3_ps[:], lhsT=ech[:, tt * D:(tt + 1) * D],
                        rhs=wv_h[:, t, :],
                        start=(t == 0), stop=(t == P - 1))

            # ============== memory-bank base sums (post-AG) ==============
            rm_pos = sb.tile([P, 8], BF16)
            nc.vector.tensor_scalar(out=rm_pos[:], in0=iotam[:],
                                    scalar1=n3b[:, 1:2], scalar2=None,
                                    op0=Alu.is_ge)
            rm_neg = sb.tile([P, 8], BF16)
            nc.vector.tensor_scalar(out=rm_neg[:], in0=iotam[:],
                                    scalar1=n3b[:, 2:3], scalar2=-1.0,
                                    op0=Alu.is_ge, op1=Alu.mult)
            bpn_ps = ps.tile([P, 1], F32)
            for t in range(8):
                nc.tensor.matmul(out=bpn_ps[:], lhsT=pmem[:, t * D:(t + 1) * D],
                                 rhs=rm_pos[:, t:t + 1],
                                 start=(t == 0), stop=False)
            for t in range(8):
                nc.tensor.matmul(out=bpn_ps[:], lhsT=nmem[:, t * D:(t + 1) * D],
                                 rhs=rm_neg[:, t:t + 1],
                                 start=False, stop=(t == 7))

            # ============== per-core partial outputs ==============
            # col0 = anchor-weighted sum partial, col1 = (pos-neg+banks)
            # partial, [0,2] = n_anchor.  The 10-flop global dot + relu
            # happens on the host as part of the gather/unshard step.
            out3 = sb.tile([P, 3], F32)
            s3_sb = sb.tile([P, 3], F32)
            nc.scalar.copy(s3_sb[:], s3_ps[:])
            nc.vector.tensor_copy(out3[:, 0:1], s3_sb[:, 0:1])
            sv = sb.tile([P, 1], F32)
            nc.vector.tensor_tensor(out=sv[:], in0=s3_sb[:, 1:2],
                                    in1=s3_sb[:, 2:3], op=Alu.subtract)
            nc.vector.scalar_tensor_tensor(
                out=out3[:, 1:2], in0=bpn_ps[:], scalar=pid0[:, 0:1],
                in1=sv[:], op0=Alu.mult, op1=Alu.add)
            nc.vector.memset(out3[:, 2:3], 0.0)
            nc.vector.tensor_copy(out3[0:1, 2:3], cn_in[:, 3:4])
            nc.sync.dma_start(out=out_t[:], in_=out3[:])

    nc.compile()
    return nc


def _consts():
    cst = np.zeros((P, 448), np.float32)
    cst[:, 0:128] = np.eye(P, dtype=np.float32)
    cst[:, 128:256] = np.triu(np.ones((P, P), np.float32), 0)  # inclusive c<=p
    cst[:, 256:264] = (np.arange(8)[None, :] * P
                       + np.arange(P)[:, None]).astype(np.float32)
    cst[:, 264] = 1.0                       # ones column (onescol / ones8)
    cst[0, 265:393] = 1.0                   # ones row (onesrow)
    cst[0:8, 393] = np.arange(8)            # iota8 column
    cst[0, 394:397] = [K_ANC, K_POS, K_NEG]
    cst[0, 408] = MARGIN
    # x24[s, i*3+x] = sender id whose data lands in receiver s's slot i.
    # The ucode's XOR-relative routing lands cross-die deltas (bit 2 set)
    # on the D2D diagonal: effective delta = i^2 for i in 4..7 (measured).
    delta = np.array([0, 1, 2, 3, 6, 7, 4, 5])
    xor_tab = np.arange(8)[:, None] ^ delta[None, :]
    cst[0:8, 416:440] = np.repeat(xor_tab, 3, axis=1).astype(np.float32)
    return dict(c_all=cst)


def _shard(preds, embeddings, fsss_gts, pos_memory, neg_memory):
    consts = _consts()
    pmem_pad = np.zeros((1024, D), np.float32)
    pmem_pad[:MEM] = pos_memory
    nmem_pad = np.zeros((1024, D), np.float32)
    nmem_pad[:MEM] = neg_memory
    pmem_h = np.ascontiguousarray(
        pmem_pad.reshape(8, P, D).transpose(1, 0, 2)).reshape(P, 8 * D)
    pmem_h = pmem_h.astype(ml_dtypes.bfloat16)
    nmem_h = np.ascontiguousarray(
        nmem_pad.reshape(8, P, D).transpose(1, 0, 2)).reshape(P, 8 * D)
    nmem_h = nmem_h.astype(ml_dtypes.bfloat16)

    in_maps = []
    for b in range(N_CORES):
        # [h, w, c] with channels contiguous
        pr = np.ascontiguousarray(
            preds[b][:, ::4, ::4].transpose(1, 2, 0)).reshape(P, P * NCH)
        pr = pr.astype(ml_dtypes.bfloat16)
        fs = np.ascontiguousarray(fsss_gts[b][::4, ::4]).astype(
            ml_dtypes.bfloat16)
        # emb chunk layout: [c, w, t*128 + ch] = emb[ch, c*16 + t, w]
        eh = np.ascontiguousarray(
            embeddings[b].reshape(D, NCHUNK, TPC, P).transpose(1, 3, 2, 0)
        ).reshape(NCHUNK, P, TPC * D).astype(ml_dtypes.bfloat16)
        m = dict(p_preds=pr, p_fsss=fs, p_emb=eh,
                 p_pmem=pmem_h, p_nmem=nmem_h)
        m.update(consts)
        in_maps.append(m)
    return in_maps


_NC_CACHE = None


def _get_nc():
    global _NC_CACHE
    if _NC_CACHE is None:
        _NC_CACHE = _build()
    return _NC_CACHE


def kernel(preds, embeddings, gts, fsss_gts, pos_memory, neg_memory, **_ignored):
    preds = np.asarray(preds, dtype=np.float32)
    embeddings = np.asarray(embeddings, dtype=np.float32)
    fsss_gts = np.asarray(fsss_gts)
    pos_memory = np.asarray(pos_memory, dtype=np.float32)
    neg_memory = np.asarray(neg_memory, dtype=np.float32)
    in_maps = _shard(preds, embeddings, fsss_gts, pos_memory, neg_memory)
    res = run_bass_kernel_spmd(_get_nc(), in_maps, CORE_IDS)
    return _finalize(res)


def _finalize(res):
    """Gather/unshard: sum per-core partial vectors, final dot + relu."""
    parts = np.stack([np.asarray(res.results[c]["out"]) for c in range(N_CORES)])
    anc = parts[:, :, 0].sum(axis=0)
    diff = parts[:, :, 1].sum(axis=0)
    n_anc = max(float(parts[0, 0, 2]), 1.0)
    val = float(anc @ diff) / (n_anc * MEM) + MARGIN
    return np.float32(max(val, 0.0))


def run_traced(**inputs):
    """test.py helper: run with NTFF tracing, return (value, BassKernelResults)."""
    import os
    stitch = os.environ.get("STITCH", "0") == "1"
    in_maps = _shard(
        np.asarray(inputs["preds"], np.float32),
        np.asarray(inputs["embeddings"], np.float32),
        np.asarray(inputs["fsss_gts"]),
        np.asarray(inputs["pos_memory"], np.float32),
        np.asarray(inputs["neg_memory"], np.float32),
    )
    res = run_bass_kernel_spmd(_get_nc(), in_maps, CORE_IDS, trace=True,
                               trace_cores=CORE_IDS, stitch_traces=stitch)
    return _finalize(res), res


# revision 51
# speedup vs baseline: 2.5675x; 2.5675x over previous
"""BoundaryLoss TRN2 kernel — 8-core SPMD Bass/Tile implementation (v3).

Data-parallel over batch B=8, one batch per NeuronCore.

v3 changes over v2 (trace-driven):
  * All bulk DMAs (emb chunks, memory banks) issue immediately at program
    start, spread across tensor/gpsimd/scalar engine queues so the sync
    queue only carries the latency-critical counts path (preds halves,
    fsss, consts, pid).
  * GpSimd no longer does the per-chunk mw multiplies (which delayed its
    collective servicing); it only does half of the channel reduce_max
    early, then sits at the two collective_compute instructions.
  * preds arrives in two halves; vector and gpsimd each reduce_max one
    half concurrently -> counts AllGather triggers ~6us earlier.
  * Norm phase: squares run bf16 tensor_tensor on vector (2x DVE mode)
    for most chunks / scalar activation for the rest; the per-tile
    sum-of-squares reduce outputs bf16 (enables 2x DVE mode).  A dummy
    Sqrt at t=0 preloads the activation table so no mid-stream reload.
  * Masks live in one [128, 3*128] tile -> one row-sum reduce.
  * Post-AG path: offsets+totals in one matmul, banks folded into the
    per-core AG2 payload (pid==0 mask), AG2 carries [128,2] only.

Self-contained: hardcodes all shapes for the nn_BoundaryLoss problem
(B=8, C=21, H=W=512, D=128, h=w=128, MEM=1000).
"""

import ml_dtypes
import numpy as np

import concourse.bass as bass
import concourse.bacc as bacc
import concourse.mybir as mybir
import concourse.tile as tile
from concourse.bass_utils import run_bass_kernel_spmd
from concourse.tile_rust import add_dep_helper

F32 = mybir.dt.float32
BF16 = mybir.dt.bfloat16
I32 = mybir.dt.int32
U32 = mybir.dt.uint32
Alu = mybir.AluOpType
AX = mybir.AxisListType
ACT = mybir.ActivationFunctionType

N_CORES = 8
CORE_IDS = list(range(N_CORES))

# The tile scheduling simulator is single-core: it never sees the remote
# semaphore increments produced by peer cores' remote DMAs, so a wait on
# them would be flagged as a deadlock.  Pre-seed those semaphores in the
# simulator only — the emitted hardware program is unchanged.
_P2P_PRESEED = []  # list of (SemaphoreHandle, count)


def _patch_coresim():
    import concourse.tile as _tile
    from concourse.bass import create_sync_update as _csu
    if getattr(_tile, "_p2p_patched", False):
        return
    _orig = _tile.CoreSim

    class _P2PCoreSim(_orig):
        def simulate(self, *a, **k):
            for sem, cnt in _P2P_PRESEED:
                try:
                    self.update_semaphore(
                        _csu(sem, cnt, None, True), instruction=None)
                except Exception:
                    pass
            return super().simulate(*a, **k)

    _tile.CoreSim = _P2PCoreSim
    _tile._p2p_patched = True
P = 128          # partitions / feature spatial dim
NCH = 21         # pred classes
D = 128          # embedding dim
MEM = 1000
K_ANC, K_POS, K_NEG = MEM // 10, MEM // 3, MEM // 3
MARGIN = 0.2
NCHUNK = 8       # embedding chunks per core (16 tiles of 128 positions each)
TPC = 16         # tiles per chunk


def _build():
    nc = bacc.Bacc("TRN2", target_bir_lowering=False, debug=False,
                   num_devices=N_CORES)

    preds_in = nc.declare_dram_parameter("p_preds", [P, P * NCH], BF16, isOutput=False)
    fsss_in = nc.declare_dram_parameter("p_fsss", [P, P], BF16, isOutput=False)
    emb_in = nc.declare_dram_parameter("p_emb", [NCHUNK, P, TPC * D], BF16, isOutput=False)
    pmem_in = nc.declare_dram_parameter("p_pmem", [P, 8 * D], BF16, isOutput=False)
    nmem_in = nc.declare_dram_parameter("p_nmem", [P, 8 * D], BF16, isOutput=False)
    cst_in = nc.declare_dram_parameter("c_all", [P, 448], F32, isOutput=False)

    out_t = nc.declare_dram_parameter("out", [P, 3], F32, isOutput=True)

    with tile.TileContext(nc, num_cores=N_CORES) as tc:
        with (
            tc.tile_pool(name="sb", bufs=1) as sb,
            tc.tile_pool(name="ps", bufs=1, space="PSUM") as ps,
            tc.tile_pool(name="pst", bufs=2, space="PSUM") as pst,
            tc.tile_pool(name="dr", bufs=1, space="DRAM") as dr,
        ):
            # ============== input DMAs ==============
            # sync queue: counts-critical path only
            cst = sb.tile([P, 448], F32)
            nc.sync.dma_start(out=cst[:], in_=cst_in[:])
            preds_sb = sb.tile([P, P * NCH], BF16)
            QW = 32 * NCH
            for q in range(4):
                nc.sync.dma_start(out=preds_sb[:, q * QW:(q + 1) * QW],
                                  in_=preds_in[:, q * QW:(q + 1) * QW])
            fsss_sb = sb.tile([P, P], BF16)
            nc.sync.dma_start(out=fsss_sb[:], in_=fsss_in[:])
            pid_sb = sb.tile([1, 1], U32)
            nc.sync.dma_start(out=pid_sb[:], in_=nc.partition_id_tensor[0:1, 0:1])

            # bulk DMA tiles; the dma_starts are issued after the counts
            # DMA so the tiny AllGather input never queues behind megabytes
            # of embedding descriptors (norm still finishes well inside the
            # collective's bootstrap window).
            emb_sb = [sb.tile([P, TPC * D], BF16, name=f"ech{c}")
                      for c in range(NCHUNK)]
            pmem = sb.tile([P, 8 * D], BF16)
            nmem = sb.tile([P, 8 * D], BF16)

            ident = cst[:, 0:128]
            lincl = cst[:, 128:256]
            iotam = cst[:, 256:264]
            onescol = cst[:, 264:265]
            ones8 = cst[0:8, 264:265]
            onesrow = cst[0:1, 265:393]
            ones8r = cst[0:1, 265:273]
            iota8 = cst[0:8, 393:394]
            k3row = cst[0:1, 394:397]
            margin0 = cst[0:1, 408:409]
            x24 = cst[0:8, 416:440]          # x24[s, i*3+x] = s ^ i

            # scalar: preload activation table (Sqrt) while DMAs stream
            dummy = sb.tile([1, 1], F32)
            nc.scalar.activation(out=dummy[:], in_=cst[0:1, 264:265], func=ACT.Sqrt)
            identb = sb.tile([P, P], BF16)
            nc.scalar.copy(identb[:], ident)
            linclb = sb.tile([P, P], BF16)   # inclusive upper-tri [c<=p]
            nc.scalar.copy(linclb[:], lincl)
            onescolb = sb.tile([P, 1], BF16)
            nc.scalar.copy(onescolb[:], onescol)

            # ============== mask phase ==============
            preds_re = preds_sb[:].rearrange("p (w c) -> p w c", c=NCH)
            mx = sb.tile([P, P], BF16)
            for q in range(4):
                nc.vector.reduce_max(out=mx[:, q * 32:(q + 1) * 32],
                                     in_=preds_re[:, q * 32:(q + 1) * 32, 1:NCH],
                                     axis=AX.X)
            pm = sb.tile([P, P], BF16)  # pred_mask
            nc.vector.tensor_tensor(out=pm[:], in0=mx[:], in1=preds_re[:, :, 0],
                                    op=Alu.is_gt)
            e0 = sb.tile([P, P], BF16)
            nc.vector.tensor_scalar(out=e0[:], in0=fsss_sb[:], scalar1=0.0,
                                    scalar2=None, op0=Alu.is_equal)
            e255 = sb.tile([P, P], BF16)
            nc.vector.tensor_scalar(out=e255[:], in0=fsss_sb[:], scalar1=255.0,
                                    scalar2=None, op0=Alu.is_equal)
            s01 = sb.tile([P, P], BF16)
            nc.vector.tensor_tensor(out=s01[:], in0=e0[:], in1=e255[:], op=Alu.add)
            fm = sb.tile([P, P], BF16)  # fsss_mask
            nc.vector.tensor_scalar(out=fm[:], in0=s01[:], scalar1=-1.0,
                                    scalar2=1.0, op0=Alu.mult, op1=Alu.add)
            m3 = sb.tile([P, 3 * P], BF16)  # anchor | positive | negative
            am, pom, nm = m3[:, 0:P], m3[:, P:2 * P], m3[:, 2 * P:3 * P]
            nc.vector.tensor_tensor(out=am, in0=pm[:], in1=fm[:], op=Alu.mult)
            nc.vector.tensor_tensor(out=pom, in0=fm[:], in1=am, op=Alu.subtract)
            nc.vector.tensor_tensor(out=nm, in0=pm[:], in1=e0[:], op=Alu.mult)

            # row sums -> totals -> counts AllGather input, ASAP
            rs = sb.tile([P, 3], F32)
            rs_i = nc.vector.reduce_sum(
                out=rs[:], in_=m3[:].rearrange("p (x w) -> p x w", x=3), axis=AX.X)
            tot_ps = pst.tile([1, 3], F32, tag="smallps")
            nc.tensor.matmul(out=tot_ps[:], lhsT=onescol[:], rhs=rs[:],
                             start=True, stop=True)
            tot_sb = sb.tile([1, 4], F32)
            nc.vector.tensor_copy(tot_sb[:, 0:3], tot_ps[:])
            ag_in_d = dr.tile([1, 3], F32)
            ag_out_d = dr.tile([8, 3], F32, addr_space="Shared")
            agin_dma = nc.sync.dma_start(out=ag_in_d[:], in_=tot_sb[:, 0:3])
            nc.gpsimd.collective_compute(
                "AllGather", Alu.bypass, replica_groups=[CORE_IDS],
                ins=[ag_in_d[:]], outs=[ag_out_d[:]])
            for c in range(3):
                d = nc.sync.dma_start(out=emb_sb[c][:], in_=emb_in[c])
                add_dep_helper(d.ins, agin_dma.ins, reason="counts DMA first")
            for c in range(3, 8):
                d = nc.scalar.dma_start(out=emb_sb[c][:], in_=emb_in[c])
                add_dep_helper(d.ins, agin_dma.ins, reason="counts DMA first")
            nc.sync.dma_start(out=pmem[:], in_=pmem_in[:])
            nc.sync.dma_start(out=nmem[:], in_=nmem_in[:])
            cnts8 = sb.tile([8, 3], F32)
            nc.sync.dma_start(out=cnts8[:], in_=ag_out_d[:])

            # pid helpers: lower-tri column, [128,1] pid broadcast, pid==0
            pid_f = sb.tile([1, 1], F32)
            nc.vector.tensor_copy(pid_f[:], pid_sb[:])
            pidb_ps = pst.tile([P, 1], F32, tag="smallps")
            nc.tensor.matmul(out=pidb_ps[:], lhsT=onesrow[:], rhs=pid_f[:],
                             start=True, stop=True)
            pid0 = sb.tile([P, 1], F32)
            nc.vector.tensor_scalar(out=pid0[:], in0=pidb_ps[:], scalar1=1.0,
                                    scalar2=None, op0=Alu.is_lt)
            ltm = sb.tile([8, 2], F32)
            nc.vector.tensor_tensor(out=ltm[:, 0:1], in0=iota8[:],
                                    in1=pidb_ps[0:8, :], op=Alu.is_lt)
            nc.vector.tensor_copy(ltm[:, 1:2], ones8)

            # transposes: masks to [w, x*128+h] layout (pre-AG, PE)
            mT = sb.tile([P, 3 * P], BF16)
            for x in range(3):
                mt_ps = pst.tile([P, P], BF16, name="mt_ps", tag="smallps")
                nc.tensor.transpose(out=mt_ps[:], in_=m3[:, x * P:(x + 1) * P],
                                    identity=identb[:])
                nc.scalar.copy(mT[:, x * P:(x + 1) * P], mt_ps[:])

            # cumT[w, x*128+h] = excl[x,h] + sum_{w'<=w} mT[w', x*128+h]
            # built entirely on PE: column-prefix matmul + broadcast add.
            rsT_ps = pst.tile([1, 3 * P], F32, tag="rsT")
            nc.tensor.matmul(out=rsT_ps[:], lhsT=onescolb[:], rhs=mT[:],
                             start=True, stop=True)
            rsT_sb = sb.tile([1, 3 * P], F32)
            nc.vector.tensor_copy(rsT_sb[:], rsT_ps[:])
            excl_row = sb.tile([1, 3 * P], F32)
            nc.vector.memset(excl_row[:], 0.0)
            for x in range(3):
                nc.vector.tensor_tensor_scan(
                    out=excl_row[0:1, x * P + 1:(x + 1) * P],
                    data0=rsT_sb[0:1, x * P:x * P + P - 1],
                    data1=rsT_sb[0:1, x * P:x * P + P - 1],
                    initial=0.0, op0=Alu.add, op1=Alu.bypass)
            cumT_ps = ps.tile([P, 3 * P], F32)
            nc.tensor.matmul(out=cumT_ps[:], lhsT=linclb[:], rhs=mT[:],
                             start=True, stop=False)
            nc.tensor.matmul(out=cumT_ps[:], lhsT=onesrow[:], rhs=excl_row[:],
                             start=False, stop=True)

            # ============== embedding normalisation stream ==============
            # inv[w, h] = 1/sqrt(sum_d emb[w, h, d]^2)
            inv_all = sb.tile([P, P], F32)
            ss = sb.tile([P, P], BF16)       # per-tile sum of squares
            nrm = sb.tile([P, P], BF16)      # sqrt of ss
            sq = [sb.tile([P, TPC * D], BF16, name=f"sq{i}") for i in range(3)]
            SC_SQ = (2, 5, 7)                # chunks squared on scalar engine
            for c in range(NCHUNK):
                ech = emb_sb[c][:]
                sqc = sq[c % 3]
                if c in SC_SQ:
                    nc.scalar.square(sqc[:], ech)
                else:
                    sq_i = nc.vector.tensor_tensor(out=sqc[:], in0=ech, in1=ech,
                                                   op=Alu.mult)
                    if c == 0:
                        # counts path keeps vector priority over norm work
                        add_dep_helper(sq_i.ins, rs_i.ins, reason="counts first")
                ssc = ss[:, c * TPC:(c + 1) * TPC]
                with nc.allow_low_precision("sumsq rounded to bf16 (tol 2e-2)"):
                    nc.vector.reduce_sum(
                        out=ssc, in_=sqc[:].rearrange("p (t d) -> p t d", t=TPC),
                        axis=AX.X)
                if c == 3:
                    nc.scalar.activation(out=nrm[:, 0:64], in_=ss[:, 0:64],
                                         func=ACT.Sqrt)
                    nc.vector.reciprocal(inv_all[:, 0:64], nrm[:, 0:64])
            nc.scalar.activation(out=nrm[:, 64:128], in_=ss[:, 64:128],
                                 func=ACT.Sqrt)
            nc.vector.reciprocal(inv_all[:, 64:128], nrm[:, 64:128])

            # mw = mT * inv (pre-AG, vector; x-major layout like mT)
            mw = sb.tile([P, 3 * P], BF16)
            mw_last = None
            for x in range(3):
                mw_last = nc.vector.tensor_tensor(
                    out=mw[:, x * P:(x + 1) * P],
                    in0=mT[:, x * P:(x + 1) * P],
                    in1=inv_all[:], op=Alu.mult)

            # ============== post-AG: offsets, cutoffs ==============
            offs_ps = pst.tile([1, 3], F32, tag="smallps")
            nc.tensor.matmul(out=offs_ps[:], lhsT=ltm[:, 0:1], rhs=cnts8[:],
                             start=True, stop=True)
            gtot_ps = pst.tile([1, 3], F32, tag="smallps")
            nc.tensor.matmul(out=gtot_ps[:], lhsT=ltm[:, 1:2], rhs=cnts8[:],
                             start=True, stop=True)
            cn_in = sb.tile([1, 6], F32)
            cn_sub = nc.vector.tensor_tensor(out=cn_in[:, 0:3], in0=k3row[:],
                                             in1=offs_ps[:], op=Alu.subtract)
            # keep pre-AG work (recip/mw) ahead of the AG-blocked ops in the
            # vector queue — without this the scheduler parks the queue at
            # cn_sub and mw slips onto the post-AG critical path.
            add_dep_helper(cn_sub.ins, mw_last.ins, reason="mw pre-AG")
            nc.vector.tensor_tensor(out=cn_in[:, 3:6], in0=gtot_ps[:],
                                    in1=k3row[:], op=Alu.min)
            n3 = cn_in[:, 3:6]
            cn_ps = pst.tile([P, 6], F32, tag="smallps")
            nc.tensor.matmul(out=cn_ps[:], lhsT=onesrow[:], rhs=cn_in[:],
                             start=True, stop=True)
            cnb = sb.tile([P, 6], F32)
            nc.vector.tensor_copy(cnb[:], cn_ps[:])
            c3b, n3b = cnb[:, 0:3], cnb[:, 3:6]

            # selection weights (x-major like mT/cumT): wv = (cumT<=c3)*mw
            wv = sb.tile([P, 3 * P], BF16)
            for x in range(3):
                nc.vector.scalar_tensor_tensor(
                    out=wv[:, x * P:(x + 1) * P],
                    in0=cumT_ps[:, x * P:(x + 1) * P],
                    scalar=c3b[:, x:x + 1], in1=mw[:, x * P:(x + 1) * P],
                    op0=Alu.is_le, op1=Alu.mult)
            wv_h = wv[:].rearrange("p (x h) -> p h x", x=3)

            # ============== S3 stream: emb stationary, weights moving ==========
            s3_ps = ps.tile([P, 3], F32)
            for c in range(NCHUNK):
                ech = emb_sb[c][:]
                for tt in range(TPC):
                    t = c * TPC + tt
                    nc.tensor.matmul(
                        out=s3_ps[:], lhsT=ech[:, tt * D:(tt + 1) * D],
                        rhs=wv_h[:, t, :],
                        start=(t == 0), stop=(t == P - 1))

            # ============== memory-bank base sums (post-AG) ==============
            rm_pos = sb.tile([P, 8], BF16)
            nc.vector.tensor_scalar(out=rm_pos[:], in0=iotam[:],
                                    scalar1=n3b[:, 1:2], scalar2=None,
                                    op0=Alu.is_ge)
            rm_neg = sb.tile([P, 8], BF16)
            nc.vector.tensor_scalar(out=rm_neg[:], in0=iotam[:],
                                    scalar1=n3b[:, 2:3], scalar2=-1.0,
                                    op0=Alu.is_ge, op1=Alu.mult)
            bpn_ps = ps.tile([P, 1], F32)
            for t in range(8):
                nc.tensor.matmul(out=bpn_ps[:], lhsT=pmem[:, t * D:(t + 1) * D],
                                 rhs=rm_pos[:, t:t + 1],
                                 start=(t == 0), stop=False)
            for t in range(8):
                nc.tensor.matmul(out=bpn_ps[:], lhsT=nmem[:, t * D:(t + 1) * D],
                                 rhs=rm_neg[:, t:t + 1],
                                 start=False, stop=(t == 7))

            # ============== per-core partial outputs ==============
            # col0 = anchor-weighted sum partial, col1 = (pos-neg+banks)
            # partial, [0,2] = n_anchor.  The 10-flop global dot + relu
            # happens on the host as part of the gather/unshard step.
            out3 = sb.tile([P, 3], F32)
            s3_sb = sb.tile([P, 3], F32)
            nc.scalar.copy(s3_sb[:], s3_ps[:])
            nc.vector.tensor_copy(out3[:, 0:1], s3_sb[:, 0:1])
            sv = sb.tile([P, 1], F32)
            nc.vector.tensor_tensor(out=sv[:], in0=s3_sb[:, 1:2],
                                    in1=s3_sb[:, 2:3], op=Alu.subtract)
            nc.vector.scalar_tensor_tensor(
                out=out3[:, 1:2], in0=bpn_ps[:], scalar=pid0[:, 0:1],
                in1=sv[:], op0=Alu.mult, op1=Alu.add)
            nc.vector.memset(out3[:, 2:3], 0.0)
            nc.vector.tensor_copy(out3[0:1, 2:3], cn_in[:, 3:4])
            nc.sync.dma_start(out=out_t[:], in_=out3[:])

    nc.compile()
    return nc


def _consts():
    cst = np.zeros((P, 448), np.float32)
    cst[:, 0:128] = np.eye(P, dtype=np.float32)
    cst[:, 128:256] = np.triu(np.ones((P, P), np.float32), 0)  # inclusive c<=p
    cst[:, 256:264] = (np.arange(8)[None, :] * P
                       + np.arange(P)[:, None]).astype(np.float32)
    cst[:, 264] = 1.0                       # ones column (onescol / ones8)
    cst[0, 265:393] = 1.0                   # ones row (onesrow)
    cst[0:8, 393] = np.arange(8)            # iota8 column
    cst[0, 394:397] = [K_ANC, K_POS, K_NEG]
    cst[0, 408] = MARGIN
    # x24[s, i*3+x] = sender id whose data lands in receiver s's slot i.
    # The ucode's XOR-relative routing lands cross-die deltas (bit 2 set)
    # on the D2D diagonal: effective delta = i^2 for i in 4..7 (measured).
    delta = np.array([0, 1, 2, 3, 6, 7, 4, 5])
    xor_tab = np.arange(8)[:, None] ^ delta[None, :]
    cst[0:8, 416:440] = np.repeat(xor_tab, 3, axis=1).astype(np.float32)
    return dict(c_all=cst)


def _shard(preds, embeddings, fsss_gts, pos_memory, neg_memory):
    consts = _consts()
    pmem_pad = np.zeros((1024, D), np.float32)
    pmem_pad[:MEM] = pos_memory
    nmem_pad = np.zeros((1024, D), np.float32)
    nmem_pad[:MEM] = neg_memory
    pmem_h = np.ascontiguousarray(
        pmem_pad.reshape(8, P, D).transpose(1, 0, 2)).reshape(P, 8 * D)
    pmem_h = pmem_h.astype(ml_dtypes.bfloat16)
    nmem_h = np.ascontiguousarray(
        nmem_pad.reshape(8, P, D).transpose(1, 0, 2)).reshape(P, 8 * D)
    nmem_h = nmem_h.astype(ml_dtypes.bfloat16)

    in_maps = []
    for b in range(N_CORES):
        # [h, w, c] with channels contiguous
        pr = np.ascontiguousarray(
            preds[b][:, ::4, ::4].transpose(1, 2, 0)).reshape(P, P * NCH)
        pr = pr.astype(ml_dtypes.bfloat16)
        fs = np.ascontiguousarray(fsss_gts[b][::4, ::4]).astype(
            ml_dtypes.bfloat16)
        # emb chunk layout: [c, w, t*128 + ch] = emb[ch, c*16 + t, w]
        eh = np.ascontiguousarray(
            embeddings[b].reshape(D, NCHUNK, TPC, P).transpose(1, 3, 2, 0)
        ).reshape(NCHUNK, P, TPC * D).astype(ml_dtypes.bfloat16)
        m = dict(p_preds=pr, p_fsss=fs, p_emb=eh,
                 p_pmem=pmem_h, p_nmem=nmem_h)
        m.update(consts)
        in_maps.append(m)
    return in_maps


_NC_CACHE = None


def _get_nc():
    global _NC_CACHE
    if _NC_CACHE is None:
        _NC_CACHE = _build()
    return _NC_CACHE


def kernel(preds, embeddings, gts, fsss_gts, pos_memory, neg_memory, **_ignored):
    preds = np.asarray(preds, dtype=np.float32)
    embeddings = np.asarray(embeddings, dtype=np.float32)
    fsss_gts = np.asarray(fsss_gts)
    pos_memory = np.asarray(pos_memory, dtype=np.float32)
    neg_memory = np.asarray(neg_memory, dtype=np.float32)
    in_maps = _shard(preds, embeddings, fsss_gts, pos_memory, neg_memory)
    res = run_bass_kernel_spmd(_get_nc(), in_maps, CORE_IDS)
    return _finalize(res)


def _finalize(res):
    """Gather/unshard: sum per-core partial vectors, final dot + relu."""
    parts = np.stack([np.asarray(res.results[c]["out"]) for c in range(N_CORES)])
    anc = parts[:, :, 0].sum(axis=0)
    diff = parts[:, :, 1].sum(axis=0)
    n_anc = max(float(parts[0, 0, 2]), 1.0)
    val = float(anc @ diff) / (n_anc * MEM) + MARGIN
    return np.float32(max(val, 0.0))


def run_traced(**inputs):
    """test.py helper: run with NTFF tracing, return (value, BassKernelResults)."""
    import os
    stitch = os.environ.get("STITCH", "0") == "1"
    in_maps = _shard(
        np.asarray(inputs["preds"], np.float32),
        np.asarray(inputs["embeddings"], np.float32),
        np.asarray(inputs["fsss_gts"]),
        np.asarray(inputs["pos_memory"], np.float32),
        np.asarray(inputs["neg_memory"], np.float32),
    )
    res = run_bass_kernel_spmd(_get_nc(), in_maps, CORE_IDS, trace=True,
                               trace_cores=CORE_IDS, stitch_traces=stitch)
    return _finalize(res), res
